# revision 13
# baseline (speedup 1.0000x reference)
"""SpMM (GCN layer) kernel for 8 TRN2 NeuronCores — V3.

out[i] = sum_{e: row[e]==i} vals[e] * embeds[col[e]]   (N=100000, E=3.2M, d=32)

Design:
  - 1D row partition, 12500 rows/core; bf16 table replicated, chunked
    4x25000 at 256B stride (int16 gather indices, 64B payloads).
  - Main grid: first K8=8 edges of each (row, chunk) at fixed slots
    (partition row%128, cols (row//128)*8..+8 of the chunk block).
    DVE reduces k=8 -> per-chunk row-major partial [128, 98, 32] fp32,
    accumulated on-chip across chunks; one dense y write.
  - Spill edges (deg>8 per (row,chunk), ~13% of edges) go to K4=4
    groups in per-(chunk, wave) regions (wave w holds edges
    8+4w..8+4w+3). Each (chunk, wave) region is CCE scatter-added into
    a per-wave accumulator y2[w] as soon as its chunk is reduced, so
    the serial same-tensor chains overlap chunk compute; a final
    on-chip pass merges y2[*] into the main sums before the dense y
    write. Rows within one scatter call are distinct by construction
    and same-wave calls are ordered by Tile's same-tensor DRAM
    tracking — concurrent duplicate-row RMW adds lose updates on HW.
  - Pad slots: random idx, val 0; pad spill groups scatter exact zeros
    into dead rows >= 12500 (host ignores them).
  - SWDGE queue is patched post-finalize to match the DMASW lane
    (round-robin over pool DMAs in final BIR order; the ucode locks
    each lane's sem to the first queue using it).
"""

import sys

if "/opt/trn_rl_repo" not in sys.path:
    sys.path.insert(0, "/opt/trn_rl_repo")

import numpy as np
import ml_dtypes

import concourse.bass as bass
import concourse.tile as tile
from concourse import bacc, mybir, bass_utils
from concourse import ap_utils
from concourse.bass import round_up_to_multiple, exact_div

# ---------------- problem geometry (hardcoded) ----------------
N_NODES = 100000
N_EDGES = 3200000
D = 32
NC = 8
RPC = N_NODES // NC            # rows per core = 12500
CHUNK = 25000                  # table rows per chunk (int16-safe)
NCHUNKS = 4
K8 = 8                         # main slots per (row, chunk)
K4 = 4                         # spill group size
QR = 98                        # main group-rows per partition (12544/128)
MAINC = QR * K8                # 784 main cols per chunk
WCAPS = (5632, 1024, 256, 128, 128)   # spill groups per (chunk, wave)
NW = len(WCAPS)
SPILLG = sum(WCAPS)            # 7168 spill groups per chunk
SPILLC = SPILLG // 128 * K4    # 224 spill cols per chunk
COLS = MAINC + SPILLC          # 1008 cols per chunk block
SLOTS = COLS * 128             # 129024 slots per (core, chunk)
BLKS = [96] * 10 + [48]        # gather block cols (sum == COLS)
ELEM = 32
TSTEP = 128                    # table row stride elems (bf16 -> 256B)
YROWS = 12544
YSTEP = 64                     # y row stride elems (fp32 -> 256B)
MAXDEG = K8 + K4 * NW          # 28 edges per (row, chunk) supported

# per-invocation device exec time measured via in-NEFF reps differential
# (reps=3 vs reps=1 steady state, 8 cores); updated per bench round
DEVICE_TIME_NS_ESTIMATE = 1717000

_cache = {}


def _dma_gather_raw(gp, out_ap, in_ap, idxs_ap, num_idxs, num_idxs_reg,
                    elem_size, elem_step, queue_num=0):
    """dma_gather with a sub-256B payload on a 256B-strided table."""
    assert idxs_ap.dtype == mybir.dt.int16
    assert in_ap.dtype == out_ap.dtype
    assert in_ap.space == bass.MemorySpace.DRAM
    assert idxs_ap.space == bass.MemorySpace.SBUF
    assert out_ap.space == bass.MemorySpace.SBUF
    assert ap_utils.ap_is_contiguous(in_ap.ap[1:])
    assert ap_utils.ap_is_contiguous(out_ap.ap[1:])
    assert ap_utils.ap_is_contiguous(idxs_ap.ap[1:])
    assert in_ap.ap[-1][1] == out_ap.ap[-1][1] == elem_size
    assert out_ap.ap[0][1] * out_ap.ap[1][1] == round_up_to_multiple(num_idxs, 128)
    assert in_ap.ap[0][0] == elem_step
    stride_bytes = elem_step * mybir.dt.size(in_ap.dtype)
    stride_bytes_256 = exact_div(stride_bytes, 256)
    assert stride_bytes_256 < 256
    _in_ap = gp.lower_ap_dma(in_ap, for_custom_bir_dma=True)
    _idxs_ap = gp.lower_ap(idxs_ap)
    _out_ap = gp.lower_ap(out_ap)
    return gp.add_instruction(
        mybir.InstDMAGatherAnt(
            name=gp.bass.get_next_instruction_name(),
            ins=[*_in_ap, _idxs_ap, gp.lower_val_access(gp.to_reg(num_idxs_reg))],
            outs=[_out_ap],
            transpose=False,
            num_idxs=num_idxs,
            elem_size=elem_size,
            stride_bytes_256=stride_bytes_256,
            gen_mode=0,
            single_packet=False,
            queue_num=queue_num,
            sbuf_tokens_per_rank=0,
            sbuf_free_dim_per_rank=0,
            sbuf_free_dim_pad_per_rank=0,
            sbuf_byte_offset=0,
        )
    )


def _build_module(reps=1):
    """reps>1 repeats the whole body for differential device timing
    (output of the timed runs is garbage in the spill rows; only
    reps=1 is used for real results)."""
    nc = bacc.Bacc("TRN2", target_bir_lowering=False, num_swdge_queues=2)
    f32, bf16, i16 = mybir.dt.float32, mybir.dt.bfloat16, mybir.dt.int16

    tabp = nc.dram_tensor("tabp", [N_NODES, TSTEP], bf16, kind="ExternalInput")
    idxs = nc.dram_tensor("idxs", [NCHUNKS, 128, SLOTS // 16], i16,
                          kind="ExternalInput")
    vals = nc.dram_tensor("vals", [NCHUNKS, 128, COLS], bf16,
                          kind="ExternalInput")
    sidx = nc.dram_tensor("sidx", [NCHUNKS, 128, SPILLG // 16], i16,
                          kind="ExternalInput")
    y = nc.dram_tensor("y", [YROWS, YSTEP], f32, kind="ExternalOutput")
    # per-wave spill accumulators (zero-donated); wave w of every chunk
    # scatter-adds into y2[w] as soon as its chunk is reduced — calls to
    # the same wave serialize, different waves pipeline across chunks
    y2 = [nc.dram_tensor(f"y2_{w}", [YROWS, YSTEP], f32,
                         kind="ExternalOutput") for w in range(NW)]

    with tile.TileContext(nc) as tc:
        with tc.tile_pool(name="work", bufs=6) as wp, \
             tc.tile_pool(name="ve", bufs=4) as vp, \
             tc.tile_pool(name="r8", bufs=2) as r8p, \
             tc.tile_pool(name="acc", bufs=1) as accp, \
             tc.tile_pool(name="r4", bufs=4) as r4p, \
             tc.tile_pool(name="si", bufs=4) as sip:
          for _rep in range(reps):
            out_acc = accp.tile([128, QR, ELEM], f32)
            nc.vector.memset(out_acc[:], 0.0)
            red4s = []
            for c in range(NCHUNKS):
                tab_c = tabp[c * CHUNK:(c + 1) * CHUNK, :ELEM]
                red8 = r8p.tile([128, QR, ELEM], f32, tag="r8")
                red4 = r4p.tile([128, SPILLG // 128, ELEM], f32, tag="r4")
                red4s.append(red4)
                c0 = 0
                for b, ncol in enumerate(BLKS):
                    nidx = ncol * 128
                    idx_t = wp.tile([128, ncol * 8], i16, tag="idx")
                    val_t = wp.tile([128, ncol], bf16, tag="val")
                    g_t = wp.tile([128, ncol, ELEM], bf16, tag="g")
                    ve_t = vp.tile([128, ncol, ELEM], bf16, tag="ve")
                    nc.sync.dma_start(
                        out=idx_t[:], in_=idxs[c, :, c0 * 8:(c0 + ncol) * 8])
                    nc.sync.dma_start(
                        out=val_t[:], in_=vals[c, :, c0:c0 + ncol])
                    _dma_gather_raw(
                        nc.gpsimd, g_t[:], tab_c, idx_t[:],
                        num_idxs=nidx, num_idxs_reg=nidx,
                        elem_size=ELEM, elem_step=TSTEP, queue_num=0)
                    vb = val_t[:].unsqueeze(-1).broadcast_to((128, ncol, ELEM))
                    nc.scalar.activation(
                        out=ve_t[:], in_=vb,
                        func=mybir.ActivationFunctionType.Copy)
                    nc.vector.tensor_tensor(
                        out=g_t[:], in0=g_t[:], in1=ve_t[:],
                        op=mybir.AluOpType.mult)
                    # reduces: main part (k=8) and/or spill part (k=4)
                    lo, hi = c0, c0 + ncol
                    if lo < MAINC:
                        mh = min(hi, MAINC)
                        rin = g_t[:, :mh - lo, :].rearrange(
                            "p (q k) d -> p q d k", k=K8)
                        nc.vector.tensor_reduce(
                            out=red8[:, lo // K8:mh // K8, :], in_=rin,
                            axis=mybir.AxisListType.X, op=mybir.AluOpType.add)
                    if hi > MAINC:
                        sl = max(lo, MAINC)
                        rin = g_t[:, sl - lo:, :].rearrange(
                            "p (q k) d -> p q d k", k=K4)
                        nc.vector.tensor_reduce(
                            out=red4[:, (sl - MAINC) // K4:
                                     (hi - MAINC) // K4, :],
                            in_=rin,
                            axis=mybir.AxisListType.X, op=mybir.AluOpType.add)
                    c0 += ncol
                nc.vector.tensor_tensor(
                    out=out_acc[:], in0=out_acc[:], in1=red8[:],
                    op=mybir.AluOpType.add)
                si_t = sip.tile([128, SPILLG // 16], i16, tag="si")
                nc.sync.dma_start(out=si_t[:], in_=sidx[c, :, :])
                # spill combine for this chunk: wave w -> y2[w]; rows
                # distinct within a call (one spill group per (row,
                # chunk, wave)); same-wave calls across chunks are
                # ordered by Tile's same-tensor DRAM tracking
                g0 = 0
                for w, cap in enumerate(WCAPS):
                    nc.gpsimd.dma_scatter_add(
                        out_ap=y2[w][:, :ELEM],
                        in_ap=red4[:, g0 // 128:(g0 + cap) // 128, :],
                        idxs_ap=si_t[:, g0 // 16:(g0 + cap) // 16],
                        num_idxs=cap, num_idxs_reg=cap,
                        elem_size=ELEM, elem_step=YSTEP,
                        queue_num=0)
                    g0 += cap

            # merge the wave accumulators into the main sums, then one
            # dense y write
            for w in range(NW):
                yw_t = sip.tile([128, QR, ELEM], f32, tag="yw")
                nc.sync.dma_start(
                    out=yw_t[:],
                    in_=y2[w][:, :ELEM].rearrange("(q p) d -> p q d", p=128))
                nc.vector.tensor_tensor(
                    out=out_acc[:], in0=out_acc[:], in1=yw_t[:],
                    op=mybir.AluOpType.add)
            nc.sync.dma_start(
                out=y[:, :ELEM].rearrange("(q p) d -> p q d", p=128),
                in_=out_acc[:])
    nc.finalize()
    # SWDGE queue must match the DMASW lane (rr over pool DMAs in final
    # BIR order, 8 lanes; ucode locks lane sem to first user's queue).
    ctr = 0
    for blk in nc.m.functions[0].blocks:
        for inst in blk.instructions:
            if isinstance(inst,
                          (mybir.InstDMAGatherAnt, mybir.InstDMAScatterAddAnt)):
                inst.queue_num = ctr % 8 % 2
                ctr += 1
    return nc


def _wrap16(a):
    """[..., n] -> [..., 128, n//16]: wrap in 16 partitions, replicate x8."""
    n = a.shape[-1]
    w = a.reshape(*a.shape[:-1], n // 16, 16)
    w = np.moveaxis(w, -1, -2)
    return np.ascontiguousarray(
        np.broadcast_to(w[..., None, :, :],
                        (*a.shape[:-1], 8, 16, n // 16))
        .reshape(*a.shape[:-1], 128, n // 16))


def _preprocess(adj_row, adj_col, adj_vals, embeds):
    rows = np.asarray(adj_row, dtype=np.int64)
    cols = np.asarray(adj_col, dtype=np.int64)
    vals = np.asarray(adj_vals, dtype=np.float32)

    chunk = cols // CHUNK
    order = np.lexsort((chunk, rows))
    r, c, v, ch = rows[order], cols[order], vals[order], chunk[order]
    core = r // RPC
    rloc = r % RPC
    lidx = (c % CHUNK).astype(np.int16)

    # run = maximal span of one (row, chunk)
    key = r * NCHUNKS + ch
    newrun = np.empty(len(key), bool)
    newrun[0] = True
    np.not_equal(key[1:], key[:-1], out=newrun[1:])
    run_start = np.flatnonzero(newrun)
    runid = np.cumsum(newrun) - 1
    k_in = np.arange(len(key)) - run_start[runid]
    if k_in.max() >= MAXDEG:
        raise RuntimeError(f"deg {k_in.max()+1} > {MAXDEG} unsupported")

    rng = np.random.default_rng(12345)
    idx_arr = rng.integers(0, CHUNK, size=(NC, NCHUNKS, 128, COLS),
                           dtype=np.int64).astype(np.int16)
    val_arr = np.zeros((NC, NCHUNKS, 128, COLS), ml_dtypes.bfloat16)
    vb = v.astype(ml_dtypes.bfloat16)

    # main edges (k_in < K8): fixed grid positions
    m = k_in < K8
    p_m = rloc[m] % 128
    col_m = (rloc[m] // 128) * K8 + k_in[m]
    idx_arr[core[m], ch[m], p_m, col_m] = lidx[m]
    val_arr[core[m], ch[m], p_m, col_m] = vb[m]

    # spill groups: wave w holds edges [K8+4w, K8+4w+4) of a run.
    # Rank each run within its (core, chunk, wave) region by row order
    # (runs are already sorted by (core, row, chunk); rank via cumsum
    # over the wave's member runs, reset per (core, chunk)).
    s = ~m
    w_e = (k_in[s] - K8) // K4          # wave per spill edge
    k4_e = (k_in[s] - K8) % K4
    run_len = np.diff(np.append(run_start, len(key)))
    run_core = core[run_start]
    run_chunk = ch[run_start]
    run_row = rloc[run_start]
    nruns = len(run_start)

    wbase = np.concatenate(([0], np.cumsum(WCAPS)))[:-1]  # per-wave base
    # for each wave: member runs = run_len > K8 + 4w
    run_rank_w = np.zeros((NW, nruns), np.int64)
    srow = np.empty((NC, NCHUNKS, SPILLG), np.int16)
    # pads: dead rows (>= RPC), spread round-robin
    srow[:] = (RPC + np.arange(SPILLG) % (YROWS - RPC)).astype(np.int16)
    for w in range(NW):
        memb = run_len > K8 + K4 * w
        reg = run_core[memb] * NCHUNKS + run_chunk[memb]
        # rank within region: runs sorted by (core, row, chunk) ->
        # within a (core, chunk) region member order is row-sorted
        o = np.argsort(reg, kind="stable")
        cnt = np.bincount(reg, minlength=NC * NCHUNKS)
        if cnt.max() > WCAPS[w]:
            raise RuntimeError(
                f"wave {w} overflow: {cnt.max()} > {WCAPS[w]}")
        base = np.concatenate(([0], np.cumsum(cnt)))[:-1]
        rank_sorted = np.arange(memb.sum()) - base[reg[o]]
        rank = np.empty(memb.sum(), np.int64)
        rank[o] = rank_sorted
        run_rank_w[w, memb] = rank
        srow[reg[o] // NCHUNKS, reg[o] % NCHUNKS,
             wbase[w] + rank_sorted] = run_row[memb][o]

    gid = wbase[w_e] + run_rank_w[w_e, runid[s]]   # group idx in region
    p_s = gid % 128
    col_s = MAINC + (gid // 128) * K4 + k4_e
    idx_arr[core[s], ch[s], p_s, col_s] = lidx[s]
    val_arr[core[s], ch[s], p_s, col_s] = vb[s]

    tabp = np.zeros((N_NODES, TSTEP), ml_dtypes.bfloat16)
    tabp[:, :D] = np.asarray(embeds, np.float32).astype(ml_dtypes.bfloat16)

    idx_lin = np.ascontiguousarray(idx_arr.transpose(0, 1, 3, 2)).reshape(
        NC, NCHUNKS, SLOTS)
    idx_w = _wrap16(idx_lin)
    sidx_w = _wrap16(srow)

    in_maps = []
    for mi in range(NC):
        in_maps.append({
            "tabp": tabp,
            "idxs": np.ascontiguousarray(idx_w[mi]),
            "vals": np.ascontiguousarray(val_arr[mi]),
            "sidx": np.ascontiguousarray(sidx_w[mi]),
        })
    return in_maps


def _run(in_maps, trace=False):
    if "mod" not in _cache:
        _cache["mod"] = _build_module()
    nc = _cache["mod"]
    return bass_utils.run_bass_kernel_spmd(
        nc, in_maps, core_ids=list(range(NC)), trace=trace)


def kernel(adj_row, adj_col, adj_vals, embeds, _trace=False, _return_res=False):
    adj_row = np.asarray(adj_row)
    adj_col = np.asarray(adj_col)
    adj_vals = np.asarray(adj_vals)
    embeds = np.asarray(embeds)
    in_maps = _preprocess(adj_row, adj_col, adj_vals, embeds)
    res = _run(in_maps, trace=_trace)
    out = np.concatenate(
        [res.results[m]["y"][:RPC, :D] for m in range(NC)], axis=0)
    out = np.ascontiguousarray(out, dtype=np.float32)
    if _return_res:
        return out, res
    return out
